# revision 1
# baseline (speedup 1.0000x reference)
"""Trainium2 Bass kernel for nn_Dense_BinaryLayer (binary-weight dense layer).

out = x @ Wb + b, where Wb = binarize(W) in {-1, +1}.

Strategy: data-parallel over the 8 NeuronCores — each core handles 2048 rows
of x and the full (replicated) W and b; no collectives.  Per core:
  - W is loaded once, binarized on GpSimd to Wb' = (W > 2^-24) - 0.5 in
    float32r (this exactly mirrors the reference's fp32 hard-sigmoid +
    round-half-even: Wb = +1  iff  W > 2^-24 in fp32).
  - x tiles are DMA-cast to float32r, transposed 128x128 on the PE
    (pass-through transpose), and copied back from PSUM with a x2 scale
    (compensating the 0.5 scale of Wb') as the stationary operand.
  - f32r matmuls (full PE rate for free dim >= 256) accumulate in PSUM
    over the 8 k-tiles; DVE adds the broadcast bias while evicting.
"""
import sys

sys.path.insert(0, "/opt/trn_rl_repo")

import numpy as np

N_TOTAL = 16384
D_IN = 1024
D_OUT = 1024
N_CORES = 8
ROWS = N_TOTAL // N_CORES      # 2048 rows per core
P = 128
K_TILES = D_IN // P            # 8
I_TILES = ROWS // P            # 16
PAIRS = I_TILES // 2           # 8  (two row-tiles per DMA for >=1MiB transfers)
BIN_THRESH = 2.0 ** -24

_cached = {}


def _build():
    import concourse.tile as tile
    from concourse import bacc, mybir

    f32 = mybir.dt.float32
    f32r = mybir.dt.float32r

    nc = bacc.Bacc()
    x_d = nc.declare_dram_parameter("x", [ROWS, D_IN], f32, isOutput=False)
    w_d = nc.declare_dram_parameter("W", [D_IN, D_OUT], f32, isOutput=False)
    b_d = nc.declare_dram_parameter("b", [D_OUT], f32, isOutput=False)
    id_d = nc.declare_dram_parameter("ident", [P, P], f32, isOutput=False)
    o_d = nc.declare_dram_parameter("out", [ROWS, D_OUT], f32, isOutput=True)

    with tile.TileContext(nc) as tc:
        with (
            tc.tile_pool(name="const", bufs=1) as const,
            tc.tile_pool(name="wpool", bufs=1) as wpool,
            tc.tile_pool(name="xin", bufs=3) as xin,
            tc.tile_pool(name="xt", bufs=2) as xtp,
            tc.tile_pool(name="outp", bufs=3) as outp,
            tc.tile_pool(name="pst", bufs=2, space="PSUM") as pst,
            tc.tile_pool(name="pso", bufs=2, space="PSUM") as pso,
        ):
            # identity for PE transpose, in f32r (DMA-cast)
            id_r = const.tile([P, P], f32r, tag="id")
            nc.gpsimd.dma_start(id_r[:], id_d[:])

            # first x pair early so PE can start while W streams
            x_tiles = []
            t0 = xin.tile([P, 2, D_IN], f32r, tag="x")
            nc.gpsimd.dma_start(t0[:], x_d[0:256, :].rearrange("(t p) k -> p t k", p=P))
            x_tiles.append(t0)

            # W: one 4MB load, k-tile-partitioned layout [p, kt, j]
            w_raw = wpool.tile([P, K_TILES, D_OUT], f32, tag="wraw")
            nc.sync.dma_start(
                w_raw[:], w_d[:].rearrange("(kt p) j -> p kt j", p=P)
            )
            # binarize per k-tile on GpSimd: Wb' = (W > c) - 0.5 in {+-0.5}, f32r
            wb = wpool.tile([P, K_TILES, D_OUT], f32r, tag="wb")
            for kt in range(K_TILES):
                nc.gpsimd.tensor_scalar(
                    wb[:, kt, :], w_raw[:, kt, :],
                    BIN_THRESH, 0.5,
                    mybir.AluOpType.is_gt, mybir.AluOpType.subtract,
                )

            # bias broadcast to all partitions
            bb = const.tile([P, D_OUT], f32, tag="bb")
            nc.sync.dma_start(bb[:], b_d[:].unsqueeze(0).partition_broadcast(P))

            # remaining x pair loads (issued up front; pool bufs throttle)
            for pr in range(1, PAIRS):
                t = xin.tile([P, 2, D_IN], f32r, tag="x")
                nc.gpsimd.dma_start(
                    t[:],
                    x_d[pr * 256:(pr + 1) * 256, :].rearrange(
                        "(t p) k -> p t k", p=P
                    ),
                )
                x_tiles.append(t)

            def do_transposes(pr, t):
                """PE transposes for both row-tiles of pair pr; returns psum_t, xT."""
                outs = []
                for half in range(2):
                    ps_t = pst.tile([P, D_IN], f32r, tag="pst")
                    for kt in range(K_TILES):
                        nc.tensor.transpose(
                            ps_t[:, kt * P:(kt + 1) * P],
                            t[:, half, kt * P:(kt + 1) * P],
                            id_r[:],
                        )
                    # evict + x2 scale (compensates Wb' = +-0.5)
                    xT = xtp.tile([P, D_IN], f32r, tag="xt")
                    nc.vector.tensor_scalar(
                        xT[:], ps_t[:], 2.0, None, mybir.AluOpType.mult
                    )
                    outs.append(xT)
                return outs

            xT_pair = do_transposes(0, x_tiles[0])
            for pr in range(PAIRS):
                out_sb = outp.tile([P, 2, D_OUT], f32, tag="out")
                next_pair = None
                for half in range(2):
                    xT = xT_pair[half]
                    ps_o = pso.tile([P, D_OUT], f32, tag="pso")
                    for kt in range(K_TILES):
                        first = kt == 0
                        last = kt == K_TILES - 1
                        nc.tensor.matmul(
                            ps_o[:, 0:512],
                            xT[:, kt * P:(kt + 1) * P],
                            wb[:, kt, 0:512],
                            start=first, stop=last,
                        )
                        nc.tensor.matmul(
                            ps_o[:, 512:1024],
                            xT[:, kt * P:(kt + 1) * P],
                            wb[:, kt, 512:1024],
                            start=first, stop=last,
                        )
                    # transposes for the next pair between the two MM bursts,
                    # so PE never stalls on the DVE evictions
                    if half == 0 and pr + 1 < PAIRS:
                        next_pair = do_transposes(pr + 1, x_tiles[pr + 1])
                    # evict with bias add
                    nc.vector.tensor_tensor(
                        out=out_sb[:, half, :], in0=ps_o[:], in1=bb[:],
                        op=mybir.AluOpType.add,
                    )
                nc.sync.dma_start(
                    o_d[pr * 256:(pr + 1) * 256, :].rearrange(
                        "(t p) j -> p t j", p=P
                    ),
                    out_sb[:],
                )
                if next_pair is not None:
                    xT_pair = next_pair

    nc.compile()
    nc.finalize()
    return nc


def kernel(x, W, b):
    from concourse.bass_utils import run_bass_kernel_spmd

    if "nc" not in _cached:
        _cached["nc"] = _build()
    nc = _cached["nc"]

    x = np.ascontiguousarray(np.asarray(x, dtype=np.float32))
    W = np.ascontiguousarray(np.asarray(W, dtype=np.float32))
    b = np.ascontiguousarray(np.asarray(b, dtype=np.float32))
    ident = np.eye(P, dtype=np.float32)

    in_maps = [
        {
            "x": np.ascontiguousarray(x[c * ROWS:(c + 1) * ROWS]),
            "W": W,
            "b": b,
            "ident": ident,
        }
        for c in range(N_CORES)
    ]
    res = run_bass_kernel_spmd(nc, in_maps, list(range(N_CORES)))
    out = np.concatenate([res.results[c]["out"] for c in range(N_CORES)], axis=0)
    return out.astype(np.float32, copy=False)


# revision 2
# speedup vs baseline: 2.1601x; 2.1601x over previous
"""Trainium2 Bass kernel for nn_Dense_BinaryLayer (binary-weight dense layer).

out = x @ Wb + b, where Wb = binarize(W) in {-1, +1}.

Strategy: data-parallel over the 8 NeuronCores — each core handles 2048 rows
of x and the full (replicated) W and b; no collectives.  Per core:
  - W is loaded once, binarized on GpSimd to Wb' = (W > 2^-24) - 0.5 in
    float32r (this exactly mirrors the reference's fp32 hard-sigmoid +
    round-half-even: Wb = +1  iff  W > 2^-24 in fp32).
  - x tiles are DMA-cast to float32r, transposed 128x128 on the PE
    (pass-through transpose), and copied back from PSUM with a x2 scale
    (compensating the 0.5 scale of Wb') as the stationary operand.
  - f32r matmuls (full PE rate for free dim >= 256) accumulate in PSUM
    over the 8 k-tiles; DVE adds the broadcast bias while evicting.
"""
import sys

sys.path.insert(0, "/opt/trn_rl_repo")

import numpy as np

N_TOTAL = 16384
D_IN = 1024
D_OUT = 1024
N_CORES = 8
ROWS = N_TOTAL // N_CORES      # 2048 rows per core
P = 128
K_TILES = D_IN // P            # 8
I_TILES = ROWS // P            # 16
PAIRS = I_TILES // 2           # 8  (two row-tiles per DMA for >=1MiB transfers)
BIN_THRESH = 2.0 ** -24

_cached = {}


def _build():
    import concourse.tile as tile
    from concourse import bacc, mybir

    f32 = mybir.dt.float32
    f32r = mybir.dt.float32r

    nc = bacc.Bacc()
    x_d = nc.declare_dram_parameter("x", [ROWS, D_IN], f32, isOutput=False)
    w_d = nc.declare_dram_parameter("W", [D_IN, D_OUT], f32, isOutput=False)
    b_d = nc.declare_dram_parameter("b", [D_OUT], f32, isOutput=False)
    id_d = nc.declare_dram_parameter("ident", [P, P], f32, isOutput=False)
    o_d = nc.declare_dram_parameter("out", [ROWS, D_OUT], f32, isOutput=True)

    with tile.TileContext(nc) as tc:
        with (
            tc.tile_pool(name="const", bufs=1) as const,
            tc.tile_pool(name="wpool", bufs=1) as wpool,
            tc.tile_pool(name="xin", bufs=3) as xin,
            tc.tile_pool(name="xt", bufs=2) as xtp,
            tc.tile_pool(name="outp", bufs=3) as outp,
            tc.tile_pool(name="pst", bufs=2, space="PSUM") as pst,
            tc.tile_pool(name="pso", bufs=2, space="PSUM") as pso,
        ):
            # identity for PE transpose, in f32r (DMA-cast)
            id_r = const.tile([P, P], f32r, tag="id")
            nc.gpsimd.dma_start(id_r[:], id_d[:])

            # first x pair early so PE can start while W streams
            x_tiles = []
            t0 = xin.tile([P, 2, D_IN], f32r, tag="x")
            nc.gpsimd.dma_start(t0[:], x_d[0:256, :].rearrange("(t p) k -> p t k", p=P))
            x_tiles.append(t0)

            # W: one 4MB load, k-tile-partitioned layout [p, kt, j]
            w_raw = wpool.tile([P, K_TILES, D_OUT], f32, tag="wraw")
            nc.sync.dma_start(
                w_raw[:], w_d[:].rearrange("(kt p) j -> p kt j", p=P)
            )
            # binarize per k-tile on GpSimd: Wb' = (W > c) - 0.5 in {+-0.5}, f32r
            wb = wpool.tile([P, K_TILES, D_OUT], f32r, tag="wb")
            for kt in range(K_TILES):
                nc.vector.tensor_scalar(
                    wb[:, kt, :], w_raw[:, kt, :],
                    BIN_THRESH, 0.5,
                    mybir.AluOpType.is_gt, mybir.AluOpType.subtract,
                )

            # bias broadcast to all partitions
            bb = const.tile([P, D_OUT], f32, tag="bb")
            nc.sync.dma_start(bb[:], b_d[:].unsqueeze(0).partition_broadcast(P))

            # remaining x pair loads (issued up front; pool bufs throttle)
            for pr in range(1, PAIRS):
                t = xin.tile([P, 2, D_IN], f32r, tag="x")
                nc.gpsimd.dma_start(
                    t[:],
                    x_d[pr * 256:(pr + 1) * 256, :].rearrange(
                        "(t p) k -> p t k", p=P
                    ),
                )
                x_tiles.append(t)

            def do_transposes(pr, t):
                """PE transposes for both row-tiles of pair pr; returns psum_t, xT."""
                outs = []
                for half in range(2):
                    ps_t = pst.tile([P, D_IN], f32r, tag="pst")
                    for kt in range(K_TILES):
                        nc.tensor.transpose(
                            ps_t[:, kt * P:(kt + 1) * P],
                            t[:, half, kt * P:(kt + 1) * P],
                            id_r[:],
                        )
                    # evict + x2 scale (compensates Wb' = +-0.5)
                    xT = xtp.tile([P, D_IN], f32r, tag="xt")
                    nc.vector.tensor_scalar(
                        xT[:], ps_t[:], 2.0, None, mybir.AluOpType.mult
                    )
                    outs.append(xT)
                return outs

            xT_pair = do_transposes(0, x_tiles[0])
            for pr in range(PAIRS):
                out_sb = outp.tile([P, 2, D_OUT], f32, tag="out")
                next_pair = None
                for half in range(2):
                    xT = xT_pair[half]
                    ps_o = pso.tile([P, D_OUT], f32, tag="pso")
                    for kt in range(K_TILES):
                        first = kt == 0
                        last = kt == K_TILES - 1
                        nc.tensor.matmul(
                            ps_o[:, 0:512],
                            xT[:, kt * P:(kt + 1) * P],
                            wb[:, kt, 0:512],
                            start=first, stop=last,
                        )
                        nc.tensor.matmul(
                            ps_o[:, 512:1024],
                            xT[:, kt * P:(kt + 1) * P],
                            wb[:, kt, 512:1024],
                            start=first, stop=last,
                        )
                    # transposes for the next pair between the two MM bursts,
                    # so PE never stalls on the DVE evictions
                    if half == 0 and pr + 1 < PAIRS:
                        next_pair = do_transposes(pr + 1, x_tiles[pr + 1])
                    # evict with bias add
                    nc.vector.tensor_tensor(
                        out=out_sb[:, half, :], in0=ps_o[:], in1=bb[:],
                        op=mybir.AluOpType.add,
                    )
                nc.sync.dma_start(
                    o_d[pr * 256:(pr + 1) * 256, :].rearrange(
                        "(t p) j -> p t j", p=P
                    ),
                    out_sb[:],
                )
                if next_pair is not None:
                    xT_pair = next_pair

    nc.compile()
    nc.finalize()
    return nc


def kernel(x, W, b):
    from concourse.bass_utils import run_bass_kernel_spmd

    if "nc" not in _cached:
        _cached["nc"] = _build()
    nc = _cached["nc"]

    x = np.ascontiguousarray(np.asarray(x, dtype=np.float32))
    W = np.ascontiguousarray(np.asarray(W, dtype=np.float32))
    b = np.ascontiguousarray(np.asarray(b, dtype=np.float32))
    ident = np.eye(P, dtype=np.float32)

    in_maps = [
        {
            "x": np.ascontiguousarray(x[c * ROWS:(c + 1) * ROWS]),
            "W": W,
            "b": b,
            "ident": ident,
        }
        for c in range(N_CORES)
    ]
    res = run_bass_kernel_spmd(nc, in_maps, list(range(N_CORES)))
    out = np.concatenate([res.results[c]["out"] for c in range(N_CORES)], axis=0)
    return out.astype(np.float32, copy=False)
